# revision 1
# baseline (speedup 1.0000x reference)
"""Barycentric-coordinates KNN kernel for Trainium2 (8 NeuronCores).

Pipeline (per core = one (batch, half-of-V) pair; 8 cores cover 4 batches x 2 halves):
  Phase 1 (device): negated squared distances via TensorE matmul rows
    [-2q,1]x[p,|p|^2] fused with ACT bias/negate; per-64-column-chunk top-8
    values+indices via DVE max8/max_index -> 512 candidates per query row.
  Host: exact top-33 merge (value desc, index asc), neighbor-coordinate
    gather, SHOT weight normalization (no per-partition gather exists on-chip).
  Phase 2 (device): weighted 3x3 covariance (fused multiply-accumulate),
    closed-form eigensolver (Newton on the characteristic cubic + cross
    products), SHOT sign disambiguation, tangent-plane log map, template-cell
    nearest-3 selection via bit-packed keys (dist^2 mantissa | k-slot) and
    max8, onehot payload extraction, barycentric weights.
  Host: decode k-slots from packed keys, pidx = nbr_idx[closest], assemble
    (4, 4096, 5, 8, 3, 2) output.
"""
import sys

sys.path.insert(0, "/opt/trn_rl_repo")

import numpy as np
from contextlib import ExitStack

import concourse.bass as bass
import concourse.mybir as mybir
import concourse.tile as tile
from concourse.bass_utils import run_bass_kernel_spmd
from concourse.tile import ScopedClock

f32 = np.float32
AF = mybir.ActivationFunctionType
ALU = mybir.AluOpType
DT = mybir.dt

B, V, K = 4, 4096, 32
HALF = V // 2            # queries per core
NT = HALF // 128         # 16 v-tiles per core
NCHUNK = 64              # phase-1 chunk count (chunk width 64)
CAND = NCHUNK * 8        # 512 candidates per row
R, A = 5, 8
NCELL = R * A            # 40 template cells
EPS = 1e-8

# ---------------------------------------------------------------------------
# Tile-framework workaround: walrus rejects instructions carrying more than a
# couple of sync waits. Spread extras across single-wait NOPs.
# ---------------------------------------------------------------------------


def _patched_drain_and_barrier(self, tick_clock, wait_clock):
    probe = self.nc.sync.nop(nofuse=True)
    wait_clock.add_sem_waits(probe.ins, ScopedClock({None: tick_clock.global_clock}))
    sync_info = probe.ins.sync_info
    waits = list(sync_info.on_wait or []) if sync_info is not None else []
    if len(waits) > 1:
        sync_info.on_wait = waits[:1]
        for i in range(1, len(waits)):
            extra = self.nc.sync.nop(nofuse=True)
            if extra.ins.sync_info is None:
                extra.ins.sync_info = mybir.SyncInfo(on_wait=[waits[i]], on_update=[])
            else:
                extra.ins.sync_info.on_wait = [waits[i]]
    self.nc.sync.drain()
    self.nc.all_engine_barrier()
    assert self.sems is not None
    popped = self.nc._tile_sem_poison_stack.pop()
    assert popped is self._sem_poison
    self.nc.clear_and_free_semaphores(list(self.sems.allocated().values()))
    self.nc.all_engine_barrier()


tile.TileContext._drain_and_barrier = _patched_drain_and_barrier


def split_sync_waits(nc, max_waits=1):
    for f in nc.m.functions:
        for b in f.blocks:
            new_list = []
            dirty = False
            for ins in b.instructions:
                si = ins.sync_info
                waits = list(si.on_wait) if (si is not None and si.on_wait) else []
                if len(waits) > max_waits:
                    dirty = True
                    extras, keep = waits[:-max_waits], waits[-max_waits:]
                    for j in range(0, len(extras), max_waits):
                        nop = mybir.InstNoOp(
                            name=f"I-wsplit-{nc.next_id()}", engine=ins.engine
                        )
                        nop.sync_info = mybir.SyncInfo(
                            on_wait=extras[j : j + max_waits], on_update=[]
                        )
                        new_list.append(nop)
                    si.on_wait = keep
                new_list.append(ins)
            if dirty:
                b.instructions = new_list


# ---------------------------------------------------------------------------
# Phase 1 program
# ---------------------------------------------------------------------------


def build_phase1():
    nc = bass.Bass()
    pt4 = nc.declare_dram_parameter("pt4", [4, V], DT.float32, isOutput=False)
    qt4 = nc.declare_dram_parameter("qt4", [4, HALF], DT.float32, isOutput=False)
    nsqv = nc.declare_dram_parameter("nsqv", [128, NT], DT.float32, isOutput=False)
    candv_o = nc.declare_dram_parameter("candv", [HALF, CAND], DT.float32, isOutput=True)
    candi_o = nc.declare_dram_parameter("candi", [HALF, CAND], DT.uint32, isOutput=True)

    with tile.TileContext(nc) as tc, ExitStack() as ctx:
        cpool = ctx.enter_context(tc.tile_pool(name="const", bufs=1))
        dpool = ctx.enter_context(tc.tile_pool(name="negd2", bufs=3))
        opool = ctx.enter_context(tc.tile_pool(name="cand", bufs=4))
        ppool = ctx.enter_context(tc.tile_pool(name="psum", bufs=2, space="PSUM"))

        pt = cpool.tile([4, V], DT.float32)
        qt = cpool.tile([4, HALF], DT.float32)
        nv = cpool.tile([128, NT], DT.float32)
        cbase = cpool.tile([128, CAND], DT.uint32)
        nc.sync.dma_start(pt[:], pt4[:])
        nc.sync.dma_start(qt[:], qt4[:])
        nc.sync.dma_start(nv[:], nsqv[:])
        nc.gpsimd.iota(cbase[:], pattern=[[64, NCHUNK], [0, 8]], base=0,
                       channel_multiplier=0)

        for t in range(NT):
            negd2 = dpool.tile([128, V], DT.float32)
            for jh in range(2):
                ps = ppool.tile([128, 2048], DT.float32, space="PSUM")
                for k4 in range(4):
                    nc.tensor.matmul(
                        ps[:, k4 * 512:(k4 + 1) * 512],
                        qt[:, t * 128:(t + 1) * 128],
                        pt[:, jh * 2048 + k4 * 512: jh * 2048 + (k4 + 1) * 512],
                        start=True, stop=True,
                    )
                nc.scalar.activation(
                    negd2[:, jh * 2048:(jh + 1) * 2048], ps[:],
                    AF.Identity, bias=nv[:, t:t + 1], scale=-1.0,
                )
            cv = opool.tile([128, CAND], DT.float32, tag="cv")
            ci = opool.tile([128, CAND], DT.uint32, tag="ci")
            cg = opool.tile([128, CAND], DT.uint32, tag="cg")
            for c in range(NCHUNK):
                nc.vector.max(out=cv[:, c * 8:(c + 1) * 8],
                              in_=negd2[:, c * 64:(c + 1) * 64])
                nc.vector.max_index(out=ci[:, c * 8:(c + 1) * 8],
                                    in_max=cv[:, c * 8:(c + 1) * 8],
                                    in_values=negd2[:, c * 64:(c + 1) * 64])
            nc.gpsimd.tensor_tensor(out=cg[:], in0=ci[:], in1=cbase[:], op=ALU.add)
            nc.sync.dma_start(candv_o[t * 128:(t + 1) * 128, :], cv[:])
            nc.sync.dma_start(candi_o[t * 128:(t + 1) * 128, :], cg[:])

    split_sync_waits(nc)
    return nc


# ---------------------------------------------------------------------------
# Phase 2 program
# ---------------------------------------------------------------------------


def _register_consts(nc, values):
    for value in values:
        t = nc.alloc_sbuf_tensor(f"const-float32-{value}", [128, 1], DT.float32)
        nc.gpsimd.memset(t.ap(), value)
        nc.const_aps.aps[(DT.float32, value)] = t.ap()
    nc.all_engine_barrier()


def build_phase2():
    nc = bass.Bass()
    _register_consts(nc, [0.5])
    ngh_i = nc.declare_dram_parameter("ngh", [HALF, 96], DT.float32, isOutput=False)
    wn3_i = nc.declare_dram_parameter("wn3", [HALF, 96], DT.float32, isOutput=False)
    dd_i = nc.declare_dram_parameter("dd", [HALF, K], DT.float32, isOutput=False)
    txy_i = nc.declare_dram_parameter("txy", [128, 2 * NCELL], DT.float32, isOutput=False)
    w3_o = nc.declare_dram_parameter("w3o", [HALF, 3, NCELL], DT.float32, isOutput=True)
    m3_o = nc.declare_dram_parameter("m3o", [HALF, NCELL, 3], DT.float32, isOutput=True)

    with tile.TileContext(nc) as tc, ExitStack() as ctx:
        cp = ctx.enter_context(tc.tile_pool(name="const", bufs=1))
        sp = ctx.enter_context(tc.tile_pool(name="scratch", bufs=2))
        bp = ctx.enter_context(tc.tile_pool(name="bc", bufs=2))

        NGH = cp.tile([128, NT, 96], DT.float32)
        WN3 = cp.tile([128, NT, 96], DT.float32)
        DD = cp.tile([128, NT, K], DT.float32)
        TXY = cp.tile([128, 2 * NCELL], DT.float32)
        nc.sync.dma_start(NGH[:], ngh_i[:].rearrange("(t p) c -> p t c", p=128))
        nc.sync.dma_start(WN3[:], wn3_i[:].rearrange("(t p) c -> p t c", p=128))
        nc.sync.dma_start(DD[:], dd_i[:].rearrange("(t p) c -> p t c", p=128))
        nc.sync.dma_start(TXY[:], txy_i[:])
        TX = TXY[:, 0:NCELL]
        TY = TXY[:, NCELL:2 * NCELL]

        KIOTA = cp.tile([128, NCELL, K], DT.int32)
        nc.gpsimd.iota(KIOTA[:], pattern=[[0, NCELL], [1, K]], base=-2147483648,
                       channel_multiplier=0)
        M32 = cp.tile([128, 1], DT.int32)
        nc.vector.memset(M32[:], -32)

        _tagn = [0]

        def nt_tile(pool=cp):
            _tagn[0] += 1
            return pool.tile([128, NT], DT.float32, tag=f"nt{_tagn[0]}",
                             name=f"nt{_tagn[0]}")

        # ---- covariance accumulation ----
        CXX, CXY, CXZ, CYY, CYZ, CZZ = [nt_tile() for _ in range(6)]
        cov_dsts = {"xx": CXX, "xy": CXY, "xz": CXZ, "yy": CYY, "yz": CYZ, "zz": CZZ}
        pairs = [("xx", 0, 0), ("xy", 0, 1), ("xz", 0, 2),
                 ("yy", 1, 1), ("yz", 1, 2), ("zz", 2, 2)]
        for t in range(NT):
            nw = sp.tile([128, 96], DT.float32, tag="nw")
            nc.vector.tensor_tensor(out=nw[:], in0=NGH[:, t, :], in1=WN3[:, t, :],
                                    op=ALU.mult)
            for nmq, a, b in pairs:
                junk = sp.tile([128, K], DT.float32, tag="covjunk")
                nc.vector.scalar_tensor_tensor(
                    out=junk[:], in0=NGH[:, t, a * K:(a + 1) * K], scalar=1.0,
                    in1=nw[:, b * K:(b + 1) * K], op0=ALU.mult, op1=ALU.mult,
                    accum_out=cov_dsts[nmq][:, t:t + 1])

        # ---- eigensolver on (128, NT) ----
        def tt(dst, a, bb, op):
            nc.vector.tensor_tensor(out=dst[:], in0=a[:], in1=bb[:], op=op)

        def sq_act(dst, a):
            nc.scalar.activation(dst[:], a[:], AF.Square)

        Q = nt_tile()
        tt(Q, CXX, CYY, ALU.add)
        tt(Q, Q, CZZ, ALU.add)
        nc.vector.tensor_scalar_mul(Q[:], Q[:], 1.0 / 3.0)
        BXX, BYY, BZZ = nt_tile(), nt_tile(), nt_tile()
        tt(BXX, CXX, Q, ALU.subtract)
        tt(BYY, CYY, Q, ALU.subtract)
        tt(BZZ, CZZ, Q, ALU.subtract)
        P2 = nt_tile()
        T1 = nt_tile(sp)
        sq_act(P2, BXX)
        sq_act(T1, BYY)
        tt(P2, P2, T1, ALU.add)
        sq_act(T1, BZZ)
        tt(P2, P2, T1, ALU.add)
        T2 = nt_tile(sp)
        sq_act(T1, CXY)
        sq_act(T2, CXZ)
        tt(T1, T1, T2, ALU.add)
        sq_act(T2, CYZ)
        tt(T1, T1, T2, ALU.add)
        nc.vector.tensor_scalar_mul(T1[:], T1[:], 2.0)
        tt(P2, P2, T1, ALU.add)
        PP = nt_tile()
        PPX = nt_tile()
        nc.vector.tensor_scalar_mul(PPX[:], P2[:], 1.0 / 6.0)

        def polished_sqrt(dst, x, tmp):
            # ACT Sqrt is ~7e-6; one Newton step s' = (s + x/s)/2 fixes it
            nc.scalar.activation(dst[:], x[:], AF.Sqrt)
            nc.vector.tensor_scalar_max(tmp[:], dst[:], 1e-30)
            nc.vector.reciprocal(tmp[:], tmp[:])
            nc.vector.tensor_tensor(out=tmp[:], in0=x[:], in1=tmp[:], op=ALU.mult)
            nc.vector.tensor_tensor(out=dst[:], in0=dst[:], in1=tmp[:], op=ALU.add)
            nc.vector.tensor_scalar_mul(dst[:], dst[:], 0.5)

        polished_sqrt(PP, PPX, T2)
        PINV = nt_tile()
        nc.vector.tensor_scalar_max(PINV[:], PP[:], 1e-20)
        nc.vector.reciprocal(PINV[:], PINV[:])
        NBXX, NBYY, NBZZ, NBXY, NBXZ, NBYZ = [nt_tile() for _ in range(6)]
        tt(NBXX, BXX, PINV, ALU.mult)
        tt(NBYY, BYY, PINV, ALU.mult)
        tt(NBZZ, BZZ, PINV, ALU.mult)
        tt(NBXY, CXY, PINV, ALU.mult)
        tt(NBXZ, CXZ, PINV, ALU.mult)
        tt(NBYZ, CYZ, PINV, ALU.mult)
        # det(B̂)
        DET = nt_tile()
        sq_act(T1, NBYZ)                     # byz^2
        tt(T2, NBYY, NBZZ, ALU.mult)
        tt(T2, T2, T1, ALU.subtract)
        tt(DET, NBXX, T2, ALU.mult)          # + bxx (byy bzz - byz^2)
        tt(T1, NBXY, NBZZ, ALU.mult)
        tt(T2, NBYZ, NBXZ, ALU.mult)
        tt(T1, T1, T2, ALU.subtract)
        tt(T1, NBXY, T1, ALU.mult)
        tt(DET, DET, T1, ALU.subtract)       # - bxy (bxy bzz - byz bxz)
        tt(T1, NBXY, NBYZ, ALU.mult)
        tt(T2, NBYY, NBXZ, ALU.mult)
        tt(T1, T1, T2, ALU.subtract)
        tt(T1, NBXZ, T1, ALU.mult)
        tt(DET, DET, T1, ALU.add)            # + bxz (bxy byz - byy bxz)
        R2 = nt_tile()                       # 2r = det  clamped to [-2, 2]
        nc.vector.tensor_scalar_min(R2[:], DET[:], 2.0)
        nc.vector.tensor_scalar_max(R2[:], R2[:], -2.0)

        def newton(beta0):
            BETA = nt_tile()
            nc.vector.memset(BETA[:], beta0)
            FV = nt_tile(sp)
            B2 = nt_tile(sp)
            for _ in range(8):
                sq_act(B2, BETA)                              # β²
                tt(FV, B2, BETA, ALU.mult)                    # β³
                nc.vector.scalar_tensor_tensor(
                    out=T1[:], in0=BETA[:], scalar=3.0, in1=FV[:],
                    op0=ALU.mult, op1=ALU.subtract)           # 3β - β³ ... careful sign
                # T1 = (β*3) - β³  => f = β³-3β-2r = -(T1) - 2r
                tt(T1, T1, R2, ALU.add)                       # T1 = 3β - β³ + 2r = -f
                nc.vector.tensor_scalar(out=B2[:], in0=B2[:], scalar1=3.0,
                                        scalar2=-3.0, op0=ALU.mult, op1=ALU.add)  # f' = 3β²-3
                nc.vector.tensor_scalar_max(B2[:], B2[:], 1e-8)
                nc.vector.reciprocal(B2[:], B2[:])
                tt(T1, T1, B2, ALU.mult)                      # -f/f'
                tt(BETA, BETA, T1, ALU.add)                   # β - f/f'
            return BETA

        BMAX = newton(2.2)
        BMIN = newton(-2.2)
        LMAX = nt_tile()
        LMIN = nt_tile()
        tt(LMAX, PP, BMAX, ALU.mult)
        tt(LMAX, LMAX, Q, ALU.add)
        tt(LMIN, PP, BMIN, ALU.mult)
        tt(LMIN, LMIN, Q, ALU.add)

        def evec(lam):
            # columns of A - lam I
            D0, D1, D2 = nt_tile(sp), nt_tile(sp), nt_tile(sp)
            tt(D0, CXX, lam, ALU.subtract)
            tt(D1, CYY, lam, ALU.subtract)
            tt(D2, CZZ, lam, ALU.subtract)
            m0 = (D0, CXY, CXZ)
            m1 = (CXY, D1, CYZ)
            m2 = (CXZ, CYZ, D2)

            def cross(u, v):
                rx, ry, rz = nt_tile(sp), nt_tile(sp), nt_tile(sp)
                tt(rx, u[1], v[2], ALU.mult)
                tt(T1, u[2], v[1], ALU.mult)
                tt(rx, rx, T1, ALU.subtract)
                tt(ry, u[2], v[0], ALU.mult)
                tt(T1, u[0], v[2], ALU.mult)
                tt(ry, ry, T1, ALU.subtract)
                tt(rz, u[0], v[1], ALU.mult)
                tt(T1, u[1], v[0], ALU.mult)
                tt(rz, rz, T1, ALU.subtract)
                return rx, ry, rz

            def norm2(c):
                n = nt_tile(sp)
                sq_act(n, c[0])
                sq_act(T1, c[1])
                tt(n, n, T1, ALU.add)
                sq_act(T1, c[2])
                tt(n, n, T1, ALU.add)
                return n

            c01 = cross(m0, m1)
            c02 = cross(m0, m2)
            c12 = cross(m1, m2)
            n01, n02, n12 = norm2(c01), norm2(c02), norm2(c12)
            G1, G2, G3 = nt_tile(sp), nt_tile(sp), nt_tile(sp)
            tt(G1, n01, n02, ALU.is_ge)
            tt(G2, n01, n12, ALU.is_ge)
            tt(G1, G1, G2, ALU.mult)                    # pick01
            tt(G3, n02, n12, ALU.is_ge)
            U = nt_tile(sp)
            nc.vector.tensor_scalar(out=U[:], in0=G1[:], scalar1=-1.0, scalar2=1.0,
                                    op0=ALU.mult, op1=ALU.add)   # 1 - pick01
            tt(G2, U, G3, ALU.mult)                     # pick02
            nc.vector.tensor_scalar(out=G3[:], in0=G3[:], scalar1=-1.0, scalar2=1.0,
                                    op0=ALU.mult, op1=ALU.add)   # 1 - g3
            tt(G3, U, G3, ALU.mult)                     # pick12
            out = []
            for ci in range(3):
                VC = nt_tile()
                tt(VC, c01[ci], G1, ALU.mult)
                tt(T1, c02[ci], G2, ALU.mult)
                tt(VC, VC, T1, ALU.add)
                tt(T1, c12[ci], G3, ALU.mult)
                tt(VC, VC, T1, ALU.add)
                out.append(VC)
            n2v = norm2(out)
            n = nt_tile(sp)
            polished_sqrt(n, n2v, T1)
            nc.vector.tensor_scalar_max(n[:], n[:], 1e-30)
            nc.vector.reciprocal(n[:], n[:])
            for VC in out:
                tt(VC, VC, n, ALU.mult)
            return out

        ZAX = evec(LMIN)
        XAX = evec(LMAX)

        # ---- disambiguation dots ----
        DOTX = cp.tile([128, NT, K], DT.float32)
        DOTZ = cp.tile([128, NT, K], DT.float32)
        for t in range(NT):
            for DST, AX in ((DOTX, XAX), (DOTZ, ZAX)):
                nc.vector.tensor_scalar(
                    out=DST[:, t, :], in0=NGH[:, t, 0:K], scalar1=AX[0][:, t:t + 1],
                    scalar2=None, op0=ALU.mult)
                nc.vector.scalar_tensor_tensor(
                    out=DST[:, t, :], in0=NGH[:, t, K:2 * K], scalar=AX[1][:, t:t + 1],
                    in1=DST[:, t, :], op0=ALU.mult, op1=ALU.add)
                nc.vector.scalar_tensor_tensor(
                    out=DST[:, t, :], in0=NGH[:, t, 2 * K:3 * K], scalar=AX[2][:, t:t + 1],
                    in1=DST[:, t, :], op0=ALU.mult, op1=ALU.add)

        SG = cp.tile([128, NT, K], DT.float32)
        FX = nt_tile()
        FZ = nt_tile()
        for DOT, F in ((DOTX, FX), (DOTZ, FZ)):
            nc.scalar.activation(SG[:], DOT[:], AF.Sign)
            nc.vector.tensor_reduce(out=F[:], in_=SG[:], axis=mybir.AxisListType.X,
                                    op=ALU.add)
            nc.scalar.activation(F[:], F[:], AF.Sign, bias=0.5, scale=1.0)
        for c in range(3):
            tt(XAX[c], XAX[c], FX, ALU.mult)
            tt(ZAX[c], ZAX[c], FZ, ALU.mult)
        for t in range(NT):
            nc.vector.tensor_scalar(out=DOTX[:, t, :], in0=DOTX[:, t, :],
                                    scalar1=FX[:, t:t + 1], scalar2=None, op0=ALU.mult)
        # y = cross(z, x)
        YAX = []
        for (i1, i2) in ((1, 2), (2, 0), (0, 1)):
            YC = nt_tile()
            tt(YC, ZAX[i1], XAX[i2], ALU.mult)
            tt(T1, ZAX[i2], XAX[i1], ALU.mult)
            tt(YC, YC, T1, ALU.subtract)
            YAX.append(YC)
        DOTY = cp.tile([128, NT, K], DT.float32)
        for t in range(NT):
            nc.vector.tensor_scalar(
                out=DOTY[:, t, :], in0=NGH[:, t, 0:K], scalar1=YAX[0][:, t:t + 1],
                scalar2=None, op0=ALU.mult)
            nc.vector.scalar_tensor_tensor(
                out=DOTY[:, t, :], in0=NGH[:, t, K:2 * K], scalar=YAX[1][:, t:t + 1],
                in1=DOTY[:, t, :], op0=ALU.mult, op1=ALU.add)
            nc.vector.scalar_tensor_tensor(
                out=DOTY[:, t, :], in0=NGH[:, t, 2 * K:3 * K], scalar=YAX[2][:, t:t + 1],
                in1=DOTY[:, t, :], op0=ALU.mult, op1=ALU.add)

        # ---- projections (batched over all tiles) ----
        PX = cp.tile([128, NT, K], DT.float32)
        PY = cp.tile([128, NT, K], DT.float32)
        SC = cp.tile([128, NT, K], DT.float32)
        nc.scalar.activation(PX[:], DOTX[:], AF.Square)
        nc.scalar.activation(PY[:], DOTY[:], AF.Square)
        U2 = cp.tile([128, NT, K], DT.float32)
        nc.vector.tensor_tensor(out=U2[:], in0=PX[:], in1=PY[:], op=ALU.add)
        nc.scalar.activation(SC[:], U2[:], AF.Sqrt)
        # one Newton step: s' = 0.5 (s + u/s) makes sqrt correctly-rounded-ish
        RCN = cp.tile([128, NT, K], DT.float32)
        nc.vector.tensor_scalar_max(RCN[:], SC[:], 1e-30)
        nc.vector.reciprocal(RCN[:], RCN[:])
        nc.vector.tensor_tensor(out=RCN[:], in0=U2[:], in1=RCN[:], op=ALU.mult)
        nc.vector.tensor_tensor(out=SC[:], in0=SC[:], in1=RCN[:], op=ALU.add)
        nc.vector.tensor_scalar(out=SC[:], in0=SC[:], scalar1=0.5, scalar2=EPS,
                                op0=ALU.mult, op1=ALU.add)
        nc.vector.reciprocal(SC[:], SC[:])
        nc.vector.tensor_tensor(out=SC[:], in0=SC[:], in1=DD[:], op=ALU.mult)
        nc.vector.tensor_tensor(out=PX[:], in0=DOTX[:], in1=SC[:], op=ALU.mult)
        nc.vector.tensor_tensor(out=PY[:], in0=DOTY[:], in1=SC[:], op=ALU.mult)

        # ---- BC selection per tile ----
        PSEL = [cp.tile([128, NT, NCELL], DT.float32, tag=f'psel{i}', name=f'psel{i}') for i in range(6)]
        # PSEL order: p0x p1x p2x p0y p1y p2y
        for t in range(NT):
            pxb = PX[:, t, :].rearrange("p k -> p () k").to_broadcast([128, NCELL, K])
            pyb = PY[:, t, :].rearrange("p k -> p () k").to_broadcast([128, NCELL, K])
            txb = TX.rearrange("p r -> p r ()").to_broadcast([128, NCELL, K])
            tyb = TY.rearrange("p r -> p r ()").to_broadcast([128, NCELL, K])
            DXT = bp.tile([128, NCELL, K], DT.float32, tag="dx")
            DYT = bp.tile([128, NCELL, K], DT.float32, tag="dy")
            nc.gpsimd.tensor_tensor(out=DXT[:], in0=pxb, in1=txb, op=ALU.subtract)
            nc.gpsimd.tensor_tensor(out=DYT[:], in0=pyb, in1=tyb, op=ALU.subtract)
            SQX = bp.tile([128, NCELL, K], DT.float32, tag="sqx")
            SQY = bp.tile([128, NCELL, K], DT.float32, tag="sqy")
            nc.scalar.activation(SQX[:], DXT[:], AF.Square)
            nc.scalar.activation(SQY[:], DYT[:], AF.Square)
            SS = bp.tile([128, NCELL, K], DT.float32, tag="ss", bufs=3)
            nc.gpsimd.tensor_tensor(out=SS[:], in0=SQX[:], in1=SQY[:], op=ALU.add)
            NKEY = bp.tile([128, NCELL, K], DT.float32, tag="nkey", bufs=3)
            nc.vector.scalar_tensor_tensor(
                out=NKEY[:].bitcast(DT.int32), in0=SS[:].bitcast(DT.int32),
                scalar=M32[:], in1=KIOTA[:], op0=ALU.bitwise_and,
                op1=ALU.bitwise_or)
            M8 = bp.tile([128, NCELL, 8], DT.float32, tag="m8", bufs=3)
            for ra in range(NCELL):
                nc.vector.max(out=M8[:, ra, :], in_=NKEY[:, ra, :])
            M3C = bp.tile([128, NCELL, 3], DT.float32, tag="m3c", bufs=3)
            nc.vector.tensor_copy(M3C[:], M8[:, :, 0:3])
            nc.sync.dma_start(m3_o[t * 128:(t + 1) * 128, :, :], M3C[:])
            PXE = bp.tile([128, NCELL, K], DT.float32, tag="pxe", bufs=2)
            PYE = bp.tile([128, NCELL, K], DT.float32, tag="pye", bufs=2)
            nc.vector.tensor_copy(PXE[:], pxb)
            nc.vector.tensor_copy(PYE[:], pyb)
            for s in range(3):
                OH = bp.tile([128, NCELL, K], DT.float32, tag="oh", name="OH", bufs=3)
                msb = M8[:, :, s:s + 1].to_broadcast([128, NCELL, K])
                nc.vector.tensor_tensor(out=OH[:], in0=NKEY[:], in1=msb, op=ALU.is_equal)
                MULX = bp.tile([128, NCELL, K], DT.float32, tag="mulx", name="MULX", bufs=2)
                nc.gpsimd.tensor_tensor(out=MULX[:], in0=OH[:], in1=PXE[:], op=ALU.mult)
                nc.vector.tensor_reduce(out=PSEL[s][:, t, :], in_=MULX[:],
                                        axis=mybir.AxisListType.X, op=ALU.add)
                MULY = bp.tile([128, NCELL, K], DT.float32, tag="muly", name="MULY", bufs=2)
                nc.gpsimd.tensor_tensor(out=MULY[:], in0=OH[:], in1=PYE[:], op=ALU.mult)
                nc.vector.tensor_reduce(out=PSEL[3 + s][:, t, :], in_=MULY[:],
                                        axis=mybir.AxisListType.X, op=ALU.add)

        # ---- barycentric weights (batched (128, NT, NCELL)) ----
        P0X, P1X, P2X, P0Y, P1Y, P2Y = PSEL
        shape = [128, NT, NCELL]

        def big(tag):
            return bp.tile(shape, DT.float32, tag=tag, name=tag, bufs=1)

        def tt3(dst, a, bb, op):
            nc.vector.tensor_tensor(out=dst if isinstance(dst, bass.AP) else dst[:],
                                    in0=a if isinstance(a, bass.AP) else a[:],
                                    in1=bb if isinstance(bb, bass.AP) else bb[:],
                                    op=op)

        txb2 = TX.rearrange("p r -> p () r").to_broadcast(shape)
        tyb2 = TY.rearrange("p r -> p () r").to_broadcast(shape)
        V0X, V0Y, V1X, V1Y, V2X, V2Y = [big(f"v{i}") for i in range(6)]
        tt3(V0X, P2X, P0X, ALU.subtract)
        tt3(V0Y, P2Y, P0Y, ALU.subtract)
        tt3(V1X, P1X, P0X, ALU.subtract)
        tt3(V1Y, P1Y, P0Y, ALU.subtract)
        tt3(V2X, txb2, P0X, ALU.subtract)
        tt3(V2Y, tyb2, P0Y, ALU.subtract)

        def dot2(dst, ax, ay, bx, by, tmp):
            tt3(dst, ax, bx, ALU.mult)
            tt3(tmp, ay, by, ALU.mult)
            tt3(dst, dst, tmp, ALU.add)

        # PSEL tiles are dead once V0..V2 exist; reuse them for the dot products
        TMP = PSEL[5]
        D00, D01, D02, D11, D12 = PSEL[0], PSEL[1], PSEL[2], PSEL[3], PSEL[4]
        dot2(D00, V0X, V0Y, V0X, V0Y, TMP)
        dot2(D01, V0X, V0Y, V1X, V1Y, TMP)
        dot2(D02, V0X, V0Y, V2X, V2Y, TMP)
        dot2(D11, V1X, V1Y, V1X, V1Y, TMP)
        dot2(D12, V1X, V1Y, V2X, V2Y, TMP)
        DEN = V0X  # dead after dots
        tt3(DEN, D00, D11, ALU.mult)
        tt3(TMP, D01, D01, ALU.mult)
        tt3(DEN, DEN, TMP, ALU.subtract)
        nc.vector.tensor_scalar_add(DEN[:], DEN[:], 1e-6)
        nc.vector.reciprocal(DEN[:], DEN[:])
        W2T = V0Y
        W1T = V1X
        W0T = V1Y
        tt3(W2T, D11, D02, ALU.mult)
        tt3(TMP, D01, D12, ALU.mult)
        tt3(W2T, W2T, TMP, ALU.subtract)
        tt3(W2T, W2T, DEN, ALU.mult)
        tt3(W1T, D00, D12, ALU.mult)
        tt3(TMP, D01, D02, ALU.mult)
        tt3(W1T, W1T, TMP, ALU.subtract)
        tt3(W1T, W1T, DEN, ALU.mult)
        nc.vector.tensor_tensor(out=W0T[:], in0=W2T[:], in1=W1T[:], op=ALU.add)
        nc.vector.tensor_scalar(out=W0T[:], in0=W0T[:], scalar1=-1.0, scalar2=1.0,
                                op0=ALU.mult, op1=ALU.add)
        for s, WT in enumerate((W2T, W1T, W0T)):
            nc.sync.dma_start(
                w3_o[:, s, :].rearrange("(t p) r -> p t r", p=128), WT[:])

    split_sync_waits(nc)
    return nc


# ---------------------------------------------------------------------------
# Host glue
# ---------------------------------------------------------------------------


def host_prep_phase1(vertices):
    """vertices (4, 4096, 3) -> list of 8 input maps."""
    maps = []
    for core in range(8):
        b, h = core // 2, core % 2
        verts = np.ascontiguousarray(vertices[b], dtype=f32)
        sq = (verts * verts).sum(-1, dtype=f32).astype(f32)
        pt4 = np.concatenate([verts.T, sq[None, :]], axis=0).astype(f32)
        Q = verts[h * HALF:(h + 1) * HALF]
        qt4 = np.concatenate([-2.0 * Q.T, np.ones((1, HALF), f32)], axis=0).astype(f32)
        nsq = -sq[h * HALF:(h + 1) * HALF]
        nsqv = nsq.reshape(NT, 128).T.copy()  # [p, t]
        maps.append({"pt4": pt4, "qt4": qt4, "nsqv": np.ascontiguousarray(nsqv)})
    return maps


def host_merge(candv, candi):
    """Top-33 by (value desc, index asc). -> nbr (HALF,32) int64, d (HALF,32), radius (HALF,)."""
    order = np.lexsort((candi, -candv), axis=1)[:, :40]
    vals = np.take_along_axis(candv, order, axis=1)
    idxs = np.take_along_axis(candi, order, axis=1)
    # max_index can report the same column twice when a chunk holds two
    # bitwise-equal values; de-duplicate per row, pulling later candidates up.
    dup_rows = np.where((np.diff(np.sort(idxs[:, :33], axis=1), axis=1) == 0).any(1))[0]
    for rr in dup_rows:
        _, first = np.unique(idxs[rr], return_index=True)
        keep = np.sort(first)
        vals[rr, :keep.size] = vals[rr, keep]
        idxs[rr, :keep.size] = idxs[rr, keep]
    vals, idxs = vals[:, :33], idxs[:, :33]
    d33 = np.sqrt(np.maximum(-vals, 0.0)).astype(f32)
    return idxs[:, :32].astype(np.int64), d33[:, :32], d33[:, 32]


def host_prep_phase2(vertices, template, p1_results):
    """Build phase-2 input maps + per-core nbr tables from phase-1 outputs."""
    template = np.asarray(template, f32)
    tx = template[..., 0].reshape(-1).astype(f32)
    ty = template[..., 1].reshape(-1).astype(f32)
    txy = np.ascontiguousarray(
        np.broadcast_to(np.concatenate([tx, ty])[None, :], (128, 2 * NCELL))
    ).astype(f32)
    maps, nbrs = [], []
    for core in range(8):
        b, h = core // 2, core % 2
        verts = np.ascontiguousarray(vertices[b], dtype=f32)
        cv = p1_results[core]["candv"]
        ci = p1_results[core]["candi"]
        nbr, d, radius = host_merge(cv, ci)
        Q = verts[h * HALF:(h + 1) * HALF]
        neigh = (verts[nbr] - Q[:, None, :]).astype(f32)          # (HALF, 32, 3)
        ngh = np.ascontiguousarray(neigh.transpose(0, 2, 1).reshape(HALF, 96))
        w = (radius[:, None] - d).astype(f32)
        wn = (w / (w.sum(1, keepdims=True, dtype=f32) + f32(EPS))).astype(f32)
        wn3 = np.ascontiguousarray(np.tile(wn, (1, 3)))
        maps.append({"ngh": ngh, "wn3": wn3, "dd": np.ascontiguousarray(d),
                     "txy": txy})
        nbrs.append(nbr)
    return maps, nbrs


def host_assemble(p2_results, nbrs):
    """Decode closest slots, map to global ids, build (4, 4096, 5, 8, 3, 2)."""
    out = np.zeros((B, V, R, A, 3, 2), f32)
    for core in range(8):
        b, h = core // 2, core % 2
        m3 = np.ascontiguousarray(p2_results[core]["m3o"])        # (HALF, 40, 3)
        w3 = p2_results[core]["w3o"]                              # (HALF, 3, 40)
        k3 = (m3.view(np.int32) & 31).astype(np.int64)            # (HALF, 40, 3)
        nbr = nbrs[core]                                          # (HALF, 32)
        pidx = np.take_along_axis(nbr[:, None, :].repeat(NCELL, 1), k3, axis=2)
        sl = slice(h * HALF, (h + 1) * HALF)
        out[b, sl, ..., 0] = pidx.reshape(HALF, R, A, 3).astype(f32)
        out[b, sl, ..., 1] = w3.transpose(0, 2, 1).reshape(HALF, R, A, 3)
    return out


_PROGS = {}


def _prog(name):
    if name not in _PROGS:
        _PROGS[name] = build_phase1() if name == "p1" else build_phase2()
    return _PROGS[name]


def run_phase1(vertices, trace=False):
    maps = host_prep_phase1(vertices)
    return run_bass_kernel_spmd(_prog("p1"), maps, list(range(8)), trace=trace)


def kernel(vertices, template, trace=False, _timing=None):
    vertices = np.asarray(vertices, f32)
    template = np.asarray(template, f32)
    r1 = run_bass_kernel_spmd(_prog("p1"), host_prep_phase1(vertices),
                              list(range(8)), trace=trace)
    maps2, nbrs = host_prep_phase2(vertices, template, r1.results)
    r2 = run_bass_kernel_spmd(_prog("p2"), maps2, list(range(8)), trace=trace)
    if _timing is not None:
        _timing["phase1"] = r1
        _timing["phase2"] = r2
        _timing["maps2"] = maps2
        _timing["nbrs"] = nbrs
    return host_assemble(r2.results, nbrs)


if __name__ == "__main__":
    # Phase-1 standalone check against the cached numpy emulation.
    cache = np.load("/root/problem/dev_cache/ref.npz")
    vertices = cache["vertices"]
    res = run_phase1(vertices)
    nbad = 0
    for core in range(8):
        p1 = np.load(f"/root/problem/dev_cache/p1_{core}.npz")
        rv, ri = p1["candv"], p1["candi"]
        cv = res.results[core]["candv"]
        ci = res.results[core]["candi"]
        vs_ok = np.allclose(cv, rv, rtol=0, atol=1e-6)
        nbr, d, rad = host_merge(cv, ci)
        rnbr, rd, rrad = host_merge(rv, ri)
        idx_match = (nbr == rnbr).mean()
        print(f"core {core}: candv close={vs_ok} maxdiff={np.abs(cv-rv).max():.2e} "
              f"top33 idx match={idx_match:.6f} rad diff={np.abs(rad-rrad).max():.2e}")
        nbad += (nbr != rnbr).sum()
    print("total nbr mismatches vs emulation:", nbad)



# revision 8
# speedup vs baseline: 1.3425x; 1.3425x over previous
"""Barycentric-coordinates KNN kernel for Trainium2 (8 NeuronCores).

Pipeline (per core = one (batch, half-of-V) pair; 8 cores cover 4 batches x 2 halves):
  Phase 1 (device): negated squared distances via TensorE matmul rows
    [-2q,1]x[p,|p|^2] fused with ACT bias/negate; per-64-column-chunk top-8
    values+indices via DVE max8/max_index -> 512 candidates per query row.
  Host: exact top-33 merge (value desc, index asc), neighbor-coordinate
    gather, SHOT weight normalization (no per-partition gather exists on-chip).
  Phase 2 (device): weighted 3x3 covariance (fused multiply-accumulate),
    closed-form eigensolver (Newton on the characteristic cubic + cross
    products), SHOT sign disambiguation, tangent-plane log map, template-cell
    nearest-3 selection via bit-packed keys (dist^2 mantissa | k-slot) and
    max8, onehot payload extraction, barycentric weights.
  Host: decode k-slots from packed keys, pidx = nbr_idx[closest], assemble
    (4, 4096, 5, 8, 3, 2) output.
"""
import sys

sys.path.insert(0, "/opt/trn_rl_repo")

import numpy as np
from contextlib import ExitStack

import concourse.bass as bass
import concourse.mybir as mybir
import concourse.tile as tile
from concourse.bass_utils import run_bass_kernel_spmd
from concourse.tile import ScopedClock

f32 = np.float32
AF = mybir.ActivationFunctionType
ALU = mybir.AluOpType
DT = mybir.dt

B, V, K = 4, 4096, 32
HALF = V // 2            # queries per core
NT = HALF // 128         # 16 v-tiles per core
NCHUNK = 32              # phase-1 chunk count (chunk width 128)
CHUNKW = V // NCHUNK     # 128
CAND = NCHUNK * 8        # 256 candidates per row
R, A = 5, 8
NCELL = R * A            # 40 template cells
EPS = 1e-8
N_RADIAL, N_ANGULAR = 5, 8
TEMPLATE_RADIUS = 0.09

# ---------------------------------------------------------------------------
# Tile-framework workaround: walrus rejects instructions carrying more than a
# couple of sync waits. Spread extras across single-wait NOPs.
# ---------------------------------------------------------------------------


def _patched_drain_and_barrier(self, tick_clock, wait_clock):
    probe = self.nc.sync.nop(nofuse=True)
    wait_clock.add_sem_waits(probe.ins, ScopedClock({None: tick_clock.global_clock}))
    sync_info = probe.ins.sync_info
    waits = list(sync_info.on_wait or []) if sync_info is not None else []
    if len(waits) > 1:
        sync_info.on_wait = waits[:1]
        for i in range(1, len(waits)):
            extra = self.nc.sync.nop(nofuse=True)
            if extra.ins.sync_info is None:
                extra.ins.sync_info = mybir.SyncInfo(on_wait=[waits[i]], on_update=[])
            else:
                extra.ins.sync_info.on_wait = [waits[i]]
    self.nc.sync.drain()
    self.nc.all_engine_barrier()
    assert self.sems is not None
    popped = self.nc._tile_sem_poison_stack.pop()
    assert popped is self._sem_poison
    self.nc.clear_and_free_semaphores(list(self.sems.allocated().values()))
    self.nc.all_engine_barrier()


tile.TileContext._drain_and_barrier = _patched_drain_and_barrier


def split_sync_waits(nc, max_waits=1):
    for f in nc.m.functions:
        for b in f.blocks:
            new_list = []
            dirty = False
            for ins in b.instructions:
                si = ins.sync_info
                waits = list(si.on_wait) if (si is not None and si.on_wait) else []
                if len(waits) > max_waits:
                    dirty = True
                    extras, keep = waits[:-max_waits], waits[-max_waits:]
                    for j in range(0, len(extras), max_waits):
                        nop = mybir.InstNoOp(
                            name=f"I-wsplit-{nc.next_id()}", engine=ins.engine
                        )
                        nop.sync_info = mybir.SyncInfo(
                            on_wait=extras[j : j + max_waits], on_update=[]
                        )
                        new_list.append(nop)
                    si.on_wait = keep
                new_list.append(ins)
            if dirty:
                b.instructions = new_list


# ---------------------------------------------------------------------------
# Phase 1 program
# ---------------------------------------------------------------------------


def build_phase1():
    # -d2(q, p) via one 5-row matmul: qt5 = [2x, 2y, 2z, 1, -|q|^2],
    # pt5 = [px, py, pz, -|p|^2, 1] -> PSUM = 2 q.p - |p|^2 - |q|^2 = -d^2.
    # A 7-bit chunk-local index is packed into the mantissa low bits so a
    # single MAX8 per 128-wide chunk yields (value, index) fused; the host
    # decodes idx = bits & 127.
    nc = bass.Bass()
    pt5 = nc.declare_dram_parameter("pt5", [5, V], DT.float32, isOutput=False)
    qt5 = nc.declare_dram_parameter("qt5", [5, HALF], DT.float32, isOutput=False)
    candv_o = nc.declare_dram_parameter("candv", [HALF, CAND], DT.float32, isOutput=True)

    HC = NCHUNK // 2  # chunks per half (16)

    with tile.TileContext(nc) as tc, ExitStack() as ctx:
        cpool = ctx.enter_context(tc.tile_pool(name="const", bufs=1))
        npool = ctx.enter_context(tc.tile_pool(name="nkey", bufs=3))
        opool = ctx.enter_context(tc.tile_pool(name="cand", bufs=4))
        ppool = ctx.enter_context(tc.tile_pool(name="psum", bufs=2, space="PSUM"))

        pt = cpool.tile([5, V], DT.float32)
        qt = cpool.tile([5, HALF], DT.float32)
        J7 = cpool.tile([128, 2048], DT.int32)
        M128 = cpool.tile([128, 1], DT.int32)
        nc.sync.dma_start(pt[:], pt5[:])
        nc.sync.dma_start(qt[:], qt5[:])
        nc.gpsimd.iota(J7[:], pattern=[[0, HC], [1, CHUNKW]], base=0,
                       channel_multiplier=0)
        nc.vector.memset(M128[:], -128)

        for t in range(NT):
            for jh in range(2):
                ps = ppool.tile([128, 2048], DT.float32, space="PSUM")
                for k4 in range(4):
                    nc.tensor.matmul(
                        ps[:, k4 * 512:(k4 + 1) * 512],
                        qt[:, t * 128:(t + 1) * 128],
                        pt[:, jh * 2048 + k4 * 512: jh * 2048 + (k4 + 1) * 512],
                        start=True, stop=True,
                    )
                nk = npool.tile([128, 2048], DT.float32, tag="nk")
                nc.vector.scalar_tensor_tensor(
                    out=nk[:].bitcast(DT.int32), in0=ps[:].bitcast(DT.int32),
                    scalar=M128[:], in1=J7[:], op0=ALU.bitwise_and,
                    op1=ALU.bitwise_or)
                cv = opool.tile([128, HC * 8], DT.float32, tag="cv")
                for c in range(HC):
                    nc.vector.max(out=cv[:, c * 8:(c + 1) * 8],
                                  in_=nk[:, c * CHUNKW:(c + 1) * CHUNKW])
                nc.sync.dma_start(
                    candv_o[t * 128:(t + 1) * 128, jh * HC * 8:(jh + 1) * HC * 8],
                    cv[:])

    split_sync_waits(nc)
    return nc


# ---------------------------------------------------------------------------
# Phase 2 program
# ---------------------------------------------------------------------------


def _register_consts(nc, values):
    for value in values:
        t = nc.alloc_sbuf_tensor(f"const-float32-{value}", [128, 1], DT.float32)
        nc.gpsimd.memset(t.ap(), value)
        nc.const_aps.aps[(DT.float32, value)] = t.ap()
    nc.all_engine_barrier()


def build_phase2():
    nc = bass.Bass()
    _register_consts(nc, [0.5])
    ngh_i = nc.declare_dram_parameter("ngh", [HALF, 96], DT.float32, isOutput=False)
    wn3_i = nc.declare_dram_parameter("wn3", [HALF, 96], DT.float32, isOutput=False)
    dd_i = nc.declare_dram_parameter("dd", [HALF, K], DT.float32, isOutput=False)
    txy_i = nc.declare_dram_parameter("txy", [128, 2 * NCELL], DT.float32, isOutput=False)
    w3_o = nc.declare_dram_parameter("w3o", [HALF, 3, NCELL], DT.float32, isOutput=True)
    m3_o = nc.declare_dram_parameter("m3o", [HALF, NCELL, 3], DT.float32, isOutput=True)

    with tile.TileContext(nc) as tc, ExitStack() as ctx:
        cp = ctx.enter_context(tc.tile_pool(name="const", bufs=1))
        sp = ctx.enter_context(tc.tile_pool(name="scratch", bufs=2))
        bp = ctx.enter_context(tc.tile_pool(name="bc", bufs=2))

        NGH = cp.tile([128, NT, 96], DT.float32)
        WN3 = cp.tile([128, NT, 96], DT.float32)
        DD = cp.tile([128, NT, K], DT.float32)
        TXY = cp.tile([128, 2 * NCELL], DT.float32)
        nc.sync.dma_start(NGH[:], ngh_i[:].rearrange("(t p) c -> p t c", p=128))
        nc.sync.dma_start(WN3[:], wn3_i[:].rearrange("(t p) c -> p t c", p=128))
        nc.sync.dma_start(DD[:], dd_i[:].rearrange("(t p) c -> p t c", p=128))
        nc.sync.dma_start(TXY[:], txy_i[:])
        TX = TXY[:, 0:NCELL]
        TY = TXY[:, NCELL:2 * NCELL]

        KIOTA = cp.tile([128, NCELL, K], DT.int32)
        nc.gpsimd.iota(KIOTA[:], pattern=[[0, NCELL], [1, K]], base=-2147483648,
                       channel_multiplier=0)
        M32 = cp.tile([128, 1], DT.int32)
        nc.vector.memset(M32[:], -32)

        _tagn = [0]

        def nt_tile(pool=cp):
            _tagn[0] += 1
            return pool.tile([128, NT], DT.float32, tag=f"nt{_tagn[0]}",
                             name=f"nt{_tagn[0]}")

        # ---- covariance accumulation ----
        CXX, CXY, CXZ, CYY, CYZ, CZZ = [nt_tile() for _ in range(6)]
        cov_dsts = {"xx": CXX, "xy": CXY, "xz": CXZ, "yy": CYY, "yz": CYZ, "zz": CZZ}
        pairs = [("xx", 0, 0), ("xy", 0, 1), ("xz", 0, 2),
                 ("yy", 1, 1), ("yz", 1, 2), ("zz", 2, 2)]
        for t in range(NT):
            nw = sp.tile([128, 96], DT.float32, tag="nw")
            nc.vector.tensor_tensor(out=nw[:], in0=NGH[:, t, :], in1=WN3[:, t, :],
                                    op=ALU.mult)
            for nmq, a, b in pairs:
                junk = sp.tile([128, K], DT.float32, tag="covjunk")
                nc.vector.scalar_tensor_tensor(
                    out=junk[:], in0=NGH[:, t, a * K:(a + 1) * K], scalar=1.0,
                    in1=nw[:, b * K:(b + 1) * K], op0=ALU.mult, op1=ALU.mult,
                    accum_out=cov_dsts[nmq][:, t:t + 1])

        # ---- eigensolver on (128, NT) ----
        def tt(dst, a, bb, op):
            nc.vector.tensor_tensor(out=dst[:], in0=a[:], in1=bb[:], op=op)

        def sq_act(dst, a):
            nc.scalar.activation(dst[:], a[:], AF.Square)

        Q = nt_tile()
        tt(Q, CXX, CYY, ALU.add)
        tt(Q, Q, CZZ, ALU.add)
        nc.vector.tensor_scalar_mul(Q[:], Q[:], 1.0 / 3.0)
        BXX, BYY, BZZ = nt_tile(), nt_tile(), nt_tile()
        tt(BXX, CXX, Q, ALU.subtract)
        tt(BYY, CYY, Q, ALU.subtract)
        tt(BZZ, CZZ, Q, ALU.subtract)
        P2 = nt_tile()
        T1 = nt_tile(sp)
        sq_act(P2, BXX)
        sq_act(T1, BYY)
        tt(P2, P2, T1, ALU.add)
        sq_act(T1, BZZ)
        tt(P2, P2, T1, ALU.add)
        T2 = nt_tile(sp)
        sq_act(T1, CXY)
        sq_act(T2, CXZ)
        tt(T1, T1, T2, ALU.add)
        sq_act(T2, CYZ)
        tt(T1, T1, T2, ALU.add)
        nc.vector.tensor_scalar_mul(T1[:], T1[:], 2.0)
        tt(P2, P2, T1, ALU.add)
        PP = nt_tile()
        PPX = nt_tile()
        nc.vector.tensor_scalar_mul(PPX[:], P2[:], 1.0 / 6.0)

        def polished_sqrt(dst, x, tmp):
            # ACT Sqrt is ~7e-6; one Newton step s' = (s + x/s)/2 fixes it
            nc.scalar.activation(dst[:], x[:], AF.Sqrt)
            nc.vector.tensor_scalar_max(tmp[:], dst[:], 1e-30)
            nc.vector.reciprocal(tmp[:], tmp[:])
            nc.vector.tensor_tensor(out=tmp[:], in0=x[:], in1=tmp[:], op=ALU.mult)
            nc.vector.tensor_tensor(out=dst[:], in0=dst[:], in1=tmp[:], op=ALU.add)
            nc.vector.tensor_scalar_mul(dst[:], dst[:], 0.5)

        polished_sqrt(PP, PPX, T2)
        PINV = nt_tile()
        nc.vector.tensor_scalar_max(PINV[:], PP[:], 1e-20)
        nc.vector.reciprocal(PINV[:], PINV[:])
        NBXX, NBYY, NBZZ, NBXY, NBXZ, NBYZ = [nt_tile() for _ in range(6)]
        tt(NBXX, BXX, PINV, ALU.mult)
        tt(NBYY, BYY, PINV, ALU.mult)
        tt(NBZZ, BZZ, PINV, ALU.mult)
        tt(NBXY, CXY, PINV, ALU.mult)
        tt(NBXZ, CXZ, PINV, ALU.mult)
        tt(NBYZ, CYZ, PINV, ALU.mult)
        # det(B̂)
        DET = nt_tile()
        sq_act(T1, NBYZ)                     # byz^2
        tt(T2, NBYY, NBZZ, ALU.mult)
        tt(T2, T2, T1, ALU.subtract)
        tt(DET, NBXX, T2, ALU.mult)          # + bxx (byy bzz - byz^2)
        tt(T1, NBXY, NBZZ, ALU.mult)
        tt(T2, NBYZ, NBXZ, ALU.mult)
        tt(T1, T1, T2, ALU.subtract)
        tt(T1, NBXY, T1, ALU.mult)
        tt(DET, DET, T1, ALU.subtract)       # - bxy (bxy bzz - byz bxz)
        tt(T1, NBXY, NBYZ, ALU.mult)
        tt(T2, NBYY, NBXZ, ALU.mult)
        tt(T1, T1, T2, ALU.subtract)
        tt(T1, NBXZ, T1, ALU.mult)
        tt(DET, DET, T1, ALU.add)            # + bxz (bxy byz - byy bxz)
        R2 = nt_tile()                       # 2r = det  clamped to [-2, 2]
        nc.vector.tensor_scalar_min(R2[:], DET[:], 2.0)
        nc.vector.tensor_scalar_max(R2[:], R2[:], -2.0)

        def newton(beta0):
            BETA = nt_tile()
            nc.vector.memset(BETA[:], beta0)
            FV = nt_tile(sp)
            B2 = nt_tile(sp)
            for _ in range(8):
                sq_act(B2, BETA)                              # β²
                tt(FV, B2, BETA, ALU.mult)                    # β³
                nc.vector.scalar_tensor_tensor(
                    out=T1[:], in0=BETA[:], scalar=3.0, in1=FV[:],
                    op0=ALU.mult, op1=ALU.subtract)           # 3β - β³ ... careful sign
                # T1 = (β*3) - β³  => f = β³-3β-2r = -(T1) - 2r
                tt(T1, T1, R2, ALU.add)                       # T1 = 3β - β³ + 2r = -f
                nc.vector.tensor_scalar(out=B2[:], in0=B2[:], scalar1=3.0,
                                        scalar2=-3.0, op0=ALU.mult, op1=ALU.add)  # f' = 3β²-3
                nc.vector.tensor_scalar_max(B2[:], B2[:], 1e-8)
                nc.vector.reciprocal(B2[:], B2[:])
                tt(T1, T1, B2, ALU.mult)                      # -f/f'
                tt(BETA, BETA, T1, ALU.add)                   # β - f/f'
            return BETA

        BMAX = newton(2.2)
        BMIN = newton(-2.2)
        LMAX = nt_tile()
        LMIN = nt_tile()
        tt(LMAX, PP, BMAX, ALU.mult)
        tt(LMAX, LMAX, Q, ALU.add)
        tt(LMIN, PP, BMIN, ALU.mult)
        tt(LMIN, LMIN, Q, ALU.add)

        def evec(lam):
            # columns of A - lam I
            D0, D1, D2 = nt_tile(sp), nt_tile(sp), nt_tile(sp)
            tt(D0, CXX, lam, ALU.subtract)
            tt(D1, CYY, lam, ALU.subtract)
            tt(D2, CZZ, lam, ALU.subtract)
            m0 = (D0, CXY, CXZ)
            m1 = (CXY, D1, CYZ)
            m2 = (CXZ, CYZ, D2)

            def cross(u, v):
                rx, ry, rz = nt_tile(sp), nt_tile(sp), nt_tile(sp)
                tt(rx, u[1], v[2], ALU.mult)
                tt(T1, u[2], v[1], ALU.mult)
                tt(rx, rx, T1, ALU.subtract)
                tt(ry, u[2], v[0], ALU.mult)
                tt(T1, u[0], v[2], ALU.mult)
                tt(ry, ry, T1, ALU.subtract)
                tt(rz, u[0], v[1], ALU.mult)
                tt(T1, u[1], v[0], ALU.mult)
                tt(rz, rz, T1, ALU.subtract)
                return rx, ry, rz

            def norm2(c):
                n = nt_tile(sp)
                sq_act(n, c[0])
                sq_act(T1, c[1])
                tt(n, n, T1, ALU.add)
                sq_act(T1, c[2])
                tt(n, n, T1, ALU.add)
                return n

            c01 = cross(m0, m1)
            c02 = cross(m0, m2)
            c12 = cross(m1, m2)
            n01, n02, n12 = norm2(c01), norm2(c02), norm2(c12)
            G1, G2, G3 = nt_tile(sp), nt_tile(sp), nt_tile(sp)
            tt(G1, n01, n02, ALU.is_ge)
            tt(G2, n01, n12, ALU.is_ge)
            tt(G1, G1, G2, ALU.mult)                    # pick01
            tt(G3, n02, n12, ALU.is_ge)
            U = nt_tile(sp)
            nc.vector.tensor_scalar(out=U[:], in0=G1[:], scalar1=-1.0, scalar2=1.0,
                                    op0=ALU.mult, op1=ALU.add)   # 1 - pick01
            tt(G2, U, G3, ALU.mult)                     # pick02
            nc.vector.tensor_scalar(out=G3[:], in0=G3[:], scalar1=-1.0, scalar2=1.0,
                                    op0=ALU.mult, op1=ALU.add)   # 1 - g3
            tt(G3, U, G3, ALU.mult)                     # pick12
            out = []
            for ci in range(3):
                VC = nt_tile()
                tt(VC, c01[ci], G1, ALU.mult)
                tt(T1, c02[ci], G2, ALU.mult)
                tt(VC, VC, T1, ALU.add)
                tt(T1, c12[ci], G3, ALU.mult)
                tt(VC, VC, T1, ALU.add)
                out.append(VC)
            n2v = norm2(out)
            n = nt_tile(sp)
            polished_sqrt(n, n2v, T1)
            nc.vector.tensor_scalar_max(n[:], n[:], 1e-30)
            nc.vector.reciprocal(n[:], n[:])
            for VC in out:
                tt(VC, VC, n, ALU.mult)
            return out

        ZAX = evec(LMIN)
        XAX = evec(LMAX)

        # ---- disambiguation dots ----
        DOTX = cp.tile([128, NT, K], DT.float32)
        DOTZ = cp.tile([128, NT, K], DT.float32)
        for t in range(NT):
            for DST, AX in ((DOTX, XAX), (DOTZ, ZAX)):
                nc.vector.tensor_scalar(
                    out=DST[:, t, :], in0=NGH[:, t, 0:K], scalar1=AX[0][:, t:t + 1],
                    scalar2=None, op0=ALU.mult)
                nc.vector.scalar_tensor_tensor(
                    out=DST[:, t, :], in0=NGH[:, t, K:2 * K], scalar=AX[1][:, t:t + 1],
                    in1=DST[:, t, :], op0=ALU.mult, op1=ALU.add)
                nc.vector.scalar_tensor_tensor(
                    out=DST[:, t, :], in0=NGH[:, t, 2 * K:3 * K], scalar=AX[2][:, t:t + 1],
                    in1=DST[:, t, :], op0=ALU.mult, op1=ALU.add)

        SG = cp.tile([128, NT, K], DT.float32)
        FX = nt_tile()
        FZ = nt_tile()
        for DOT, F in ((DOTX, FX), (DOTZ, FZ)):
            nc.scalar.activation(SG[:], DOT[:], AF.Sign)
            nc.vector.tensor_reduce(out=F[:], in_=SG[:], axis=mybir.AxisListType.X,
                                    op=ALU.add)
            nc.scalar.activation(F[:], F[:], AF.Sign, bias=0.5, scale=1.0)
        for c in range(3):
            tt(XAX[c], XAX[c], FX, ALU.mult)
            tt(ZAX[c], ZAX[c], FZ, ALU.mult)
        for t in range(NT):
            nc.vector.tensor_scalar(out=DOTX[:, t, :], in0=DOTX[:, t, :],
                                    scalar1=FX[:, t:t + 1], scalar2=None, op0=ALU.mult)
        # y = cross(z, x)
        YAX = []
        for (i1, i2) in ((1, 2), (2, 0), (0, 1)):
            YC = nt_tile()
            tt(YC, ZAX[i1], XAX[i2], ALU.mult)
            tt(T1, ZAX[i2], XAX[i1], ALU.mult)
            tt(YC, YC, T1, ALU.subtract)
            YAX.append(YC)
        DOTY = cp.tile([128, NT, K], DT.float32)
        for t in range(NT):
            nc.vector.tensor_scalar(
                out=DOTY[:, t, :], in0=NGH[:, t, 0:K], scalar1=YAX[0][:, t:t + 1],
                scalar2=None, op0=ALU.mult)
            nc.vector.scalar_tensor_tensor(
                out=DOTY[:, t, :], in0=NGH[:, t, K:2 * K], scalar=YAX[1][:, t:t + 1],
                in1=DOTY[:, t, :], op0=ALU.mult, op1=ALU.add)
            nc.vector.scalar_tensor_tensor(
                out=DOTY[:, t, :], in0=NGH[:, t, 2 * K:3 * K], scalar=YAX[2][:, t:t + 1],
                in1=DOTY[:, t, :], op0=ALU.mult, op1=ALU.add)

        # ---- projections (batched over all tiles) ----
        PX = cp.tile([128, NT, K], DT.float32)
        PY = cp.tile([128, NT, K], DT.float32)
        SC = cp.tile([128, NT, K], DT.float32)
        nc.scalar.activation(PX[:], DOTX[:], AF.Square)
        nc.scalar.activation(PY[:], DOTY[:], AF.Square)
        U2 = cp.tile([128, NT, K], DT.float32)
        nc.vector.tensor_tensor(out=U2[:], in0=PX[:], in1=PY[:], op=ALU.add)
        nc.scalar.activation(SC[:], U2[:], AF.Sqrt)
        # one Newton step: s' = 0.5 (s + u/s) makes sqrt correctly-rounded-ish
        RCN = cp.tile([128, NT, K], DT.float32)
        nc.vector.tensor_scalar_max(RCN[:], SC[:], 1e-30)
        nc.vector.reciprocal(RCN[:], RCN[:])
        nc.vector.tensor_tensor(out=RCN[:], in0=U2[:], in1=RCN[:], op=ALU.mult)
        nc.vector.tensor_tensor(out=SC[:], in0=SC[:], in1=RCN[:], op=ALU.add)
        nc.vector.tensor_scalar(out=SC[:], in0=SC[:], scalar1=0.5, scalar2=EPS,
                                op0=ALU.mult, op1=ALU.add)
        nc.vector.reciprocal(SC[:], SC[:])
        nc.vector.tensor_tensor(out=SC[:], in0=SC[:], in1=DD[:], op=ALU.mult)
        nc.vector.tensor_tensor(out=PX[:], in0=DOTX[:], in1=SC[:], op=ALU.mult)
        nc.vector.tensor_tensor(out=PY[:], in0=DOTY[:], in1=SC[:], op=ALU.mult)

        # ---- BC selection per tile ----
        PSEL = [cp.tile([128, NT, NCELL], DT.float32, tag=f'psel{i}', name=f'psel{i}') for i in range(6)]
        # PSEL order: p0x p1x p2x p0y p1y p2y
        for t in range(NT):
            pxb = PX[:, t, :].rearrange("p k -> p () k").to_broadcast([128, NCELL, K])
            pyb = PY[:, t, :].rearrange("p k -> p () k").to_broadcast([128, NCELL, K])
            txb = TX.rearrange("p r -> p r ()").to_broadcast([128, NCELL, K])
            tyb = TY.rearrange("p r -> p r ()").to_broadcast([128, NCELL, K])
            DXT = bp.tile([128, NCELL, K], DT.float32, tag="dx")
            DYT = bp.tile([128, NCELL, K], DT.float32, tag="dy")
            nc.gpsimd.tensor_tensor(out=DXT[:], in0=pxb, in1=txb, op=ALU.subtract)
            nc.gpsimd.tensor_tensor(out=DYT[:], in0=pyb, in1=tyb, op=ALU.subtract)
            SQX = bp.tile([128, NCELL, K], DT.float32, tag="sqx")
            SQY = bp.tile([128, NCELL, K], DT.float32, tag="sqy")
            nc.scalar.activation(SQX[:], DXT[:], AF.Square)
            nc.scalar.activation(SQY[:], DYT[:], AF.Square)
            SS = bp.tile([128, NCELL, K], DT.float32, tag="ss", bufs=3)
            nc.gpsimd.tensor_tensor(out=SS[:], in0=SQX[:], in1=SQY[:], op=ALU.add)
            NKEY = bp.tile([128, NCELL, K], DT.float32, tag="nkey", bufs=3)
            nc.vector.scalar_tensor_tensor(
                out=NKEY[:].bitcast(DT.int32), in0=SS[:].bitcast(DT.int32),
                scalar=M32[:], in1=KIOTA[:], op0=ALU.bitwise_and,
                op1=ALU.bitwise_or)
            M8 = bp.tile([128, NCELL, 8], DT.float32, tag="m8", bufs=3)
            for ra in range(NCELL):
                nc.vector.max(out=M8[:, ra, :], in_=NKEY[:, ra, :])
            M3C = bp.tile([128, NCELL, 3], DT.float32, tag="m3c", bufs=3)
            nc.vector.tensor_copy(M3C[:], M8[:, :, 0:3])
            nc.sync.dma_start(m3_o[t * 128:(t + 1) * 128, :, :], M3C[:])
            PXE = bp.tile([128, NCELL, K], DT.float32, tag="pxe", bufs=2)
            PYE = bp.tile([128, NCELL, K], DT.float32, tag="pye", bufs=2)
            nc.vector.tensor_copy(PXE[:], pxb)
            nc.vector.tensor_copy(PYE[:], pyb)
            for s in range(3):
                OH = bp.tile([128, NCELL, K], DT.float32, tag="oh", name="OH", bufs=3)
                msb = M8[:, :, s:s + 1].to_broadcast([128, NCELL, K])
                nc.vector.tensor_tensor(out=OH[:], in0=NKEY[:], in1=msb, op=ALU.is_equal)
                MULX = bp.tile([128, NCELL, K], DT.float32, tag="mulx", name="MULX", bufs=2)
                nc.gpsimd.tensor_tensor(out=MULX[:], in0=OH[:], in1=PXE[:], op=ALU.mult)
                nc.vector.tensor_reduce(out=PSEL[s][:, t, :], in_=MULX[:],
                                        axis=mybir.AxisListType.X, op=ALU.add)
                MULY = bp.tile([128, NCELL, K], DT.float32, tag="muly", name="MULY", bufs=2)
                nc.gpsimd.tensor_tensor(out=MULY[:], in0=OH[:], in1=PYE[:], op=ALU.mult)
                nc.vector.tensor_reduce(out=PSEL[3 + s][:, t, :], in_=MULY[:],
                                        axis=mybir.AxisListType.X, op=ALU.add)

        # ---- barycentric weights (batched (128, NT, NCELL)) ----
        P0X, P1X, P2X, P0Y, P1Y, P2Y = PSEL
        shape = [128, NT, NCELL]

        def big(tag):
            return bp.tile(shape, DT.float32, tag=tag, name=tag, bufs=1)

        def tt3(dst, a, bb, op):
            nc.vector.tensor_tensor(out=dst if isinstance(dst, bass.AP) else dst[:],
                                    in0=a if isinstance(a, bass.AP) else a[:],
                                    in1=bb if isinstance(bb, bass.AP) else bb[:],
                                    op=op)

        txb2 = TX.rearrange("p r -> p () r").to_broadcast(shape)
        tyb2 = TY.rearrange("p r -> p () r").to_broadcast(shape)
        V0X, V0Y, V1X, V1Y, V2X, V2Y = [big(f"v{i}") for i in range(6)]
        tt3(V0X, P2X, P0X, ALU.subtract)
        tt3(V0Y, P2Y, P0Y, ALU.subtract)
        tt3(V1X, P1X, P0X, ALU.subtract)
        tt3(V1Y, P1Y, P0Y, ALU.subtract)
        tt3(V2X, txb2, P0X, ALU.subtract)
        tt3(V2Y, tyb2, P0Y, ALU.subtract)

        def dot2(dst, ax, ay, bx, by, tmp):
            tt3(dst, ax, bx, ALU.mult)
            tt3(tmp, ay, by, ALU.mult)
            tt3(dst, dst, tmp, ALU.add)

        # PSEL tiles are dead once V0..V2 exist; reuse them for the dot products
        TMP = PSEL[5]
        D00, D01, D02, D11, D12 = PSEL[0], PSEL[1], PSEL[2], PSEL[3], PSEL[4]
        dot2(D00, V0X, V0Y, V0X, V0Y, TMP)
        dot2(D01, V0X, V0Y, V1X, V1Y, TMP)
        dot2(D02, V0X, V0Y, V2X, V2Y, TMP)
        dot2(D11, V1X, V1Y, V1X, V1Y, TMP)
        dot2(D12, V1X, V1Y, V2X, V2Y, TMP)
        DEN = V0X  # dead after dots
        tt3(DEN, D00, D11, ALU.mult)
        tt3(TMP, D01, D01, ALU.mult)
        tt3(DEN, DEN, TMP, ALU.subtract)
        nc.vector.tensor_scalar_add(DEN[:], DEN[:], 1e-6)
        nc.vector.reciprocal(DEN[:], DEN[:])
        W2T = V0Y
        W1T = V1X
        W0T = V1Y
        tt3(W2T, D11, D02, ALU.mult)
        tt3(TMP, D01, D12, ALU.mult)
        tt3(W2T, W2T, TMP, ALU.subtract)
        tt3(W2T, W2T, DEN, ALU.mult)
        tt3(W1T, D00, D12, ALU.mult)
        tt3(TMP, D01, D02, ALU.mult)
        tt3(W1T, W1T, TMP, ALU.subtract)
        tt3(W1T, W1T, DEN, ALU.mult)
        nc.vector.tensor_tensor(out=W0T[:], in0=W2T[:], in1=W1T[:], op=ALU.add)
        nc.vector.tensor_scalar(out=W0T[:], in0=W0T[:], scalar1=-1.0, scalar2=1.0,
                                op0=ALU.mult, op1=ALU.add)
        for s, WT in enumerate((W2T, W1T, W0T)):
            nc.sync.dma_start(
                w3_o[:, s, :].rearrange("(t p) r -> p t r", p=128), WT[:])

    split_sync_waits(nc)
    return nc


# ---------------------------------------------------------------------------
# Host glue
# ---------------------------------------------------------------------------


def host_prep_phase1(vertices):
    """vertices (4, 4096, 3) -> list of 8 input maps."""
    maps = []
    for core in range(8):
        b, h = core // 2, core % 2
        verts = np.ascontiguousarray(vertices[b], dtype=f32)
        sq = (verts * verts).sum(-1, dtype=f32).astype(f32)
        pt5 = np.concatenate([verts.T, -sq[None, :], np.ones((1, V), f32)],
                             axis=0).astype(f32)
        Q = verts[h * HALF:(h + 1) * HALF]
        qsq = sq[h * HALF:(h + 1) * HALF]
        qt5 = np.concatenate([2.0 * Q.T, np.ones((1, HALF), f32),
                              -qsq[None, :]], axis=0).astype(f32)
        maps.append({"pt5": pt5, "qt5": qt5})
    return maps


def host_merge(candv):
    """Decode packed candidates, take top-33 by (d2 asc, index asc).

    candv (HALF, CAND) f32: bits = (-d2 & ~127) | chunk_local_idx, column c
    belongs to chunk c // 8. -> nbr (HALF,32) int64, d (HALF,32), radius (HALF,).
    """
    bits = candv.view(np.uint32)
    j = (bits & np.uint32(127)).astype(np.int64)
    d2 = -(bits & np.uint32(0xFFFFFF80)).view(f32)
    chunk = np.arange(CAND, dtype=np.int64) // 8
    gidx = chunk[None, :] * CHUNKW + j
    order = np.lexsort((gidx, d2), axis=1)[:, :33]
    vals = np.take_along_axis(d2, order, axis=1)
    idxs = np.take_along_axis(gidx, order, axis=1)
    d33 = np.sqrt(np.maximum(vals, 0.0)).astype(f32)
    return idxs[:, :32], d33[:, :32], d33[:, 32]


def host_prep_phase2(vertices, template, p1_results):
    """Build phase-2 input maps + per-core nbr tables from phase-1 outputs."""
    template = np.asarray(template, f32)
    tx = template[..., 0].reshape(-1).astype(f32)
    ty = template[..., 1].reshape(-1).astype(f32)
    txy = np.ascontiguousarray(
        np.broadcast_to(np.concatenate([tx, ty])[None, :], (128, 2 * NCELL))
    ).astype(f32)
    maps, nbrs = [], []
    for core in range(8):
        b, h = core // 2, core % 2
        verts = np.ascontiguousarray(vertices[b], dtype=f32)
        cv = p1_results[core]["candv"]
        nbr, d, radius = host_merge(cv)
        Q = verts[h * HALF:(h + 1) * HALF]
        neigh = (verts[nbr] - Q[:, None, :]).astype(f32)          # (HALF, 32, 3)
        ngh = np.ascontiguousarray(neigh.transpose(0, 2, 1).reshape(HALF, 96))
        w = (radius[:, None] - d).astype(f32)
        wn = (w / (w.sum(1, keepdims=True, dtype=f32) + f32(EPS))).astype(f32)
        wn3 = np.ascontiguousarray(np.tile(wn, (1, 3)))
        maps.append({"ngh": ngh, "wn3": wn3, "dd": np.ascontiguousarray(d),
                     "txy": txy})
        nbrs.append(nbr)
    return maps, nbrs


def host_assemble(p2_results, nbrs):
    """Decode closest slots, map to global ids, build (4, 4096, 5, 8, 3, 2)."""
    out = np.zeros((B, V, R, A, 3, 2), f32)
    for core in range(8):
        b, h = core // 2, core % 2
        m3 = np.ascontiguousarray(p2_results[core]["m3o"])        # (HALF, 40, 3)
        w3 = p2_results[core]["w3o"]                              # (HALF, 3, 40)
        k3 = (m3.view(np.int32) & 31).astype(np.int64)            # (HALF, 40, 3)
        nbr = nbrs[core]                                          # (HALF, 32)
        pidx = np.take_along_axis(nbr[:, None, :].repeat(NCELL, 1), k3, axis=2)
        sl = slice(h * HALF, (h + 1) * HALF)
        out[b, sl, ..., 0] = pidx.reshape(HALF, R, A, 3).astype(f32)
        out[b, sl, ..., 1] = w3.transpose(0, 2, 1).reshape(HALF, R, A, 3)
    return out


_PROGS = {}


def _prog(name):
    if name not in _PROGS:
        _PROGS[name] = build_phase1() if name == "p1" else build_phase2()
    return _PROGS[name]


def run_phase1(vertices, trace=False):
    maps = host_prep_phase1(vertices)
    return run_bass_kernel_spmd(_prog("p1"), maps, list(range(8)), trace=trace)


def kernel(vertices, template, trace=False, _timing=None):
    vertices = np.asarray(vertices, f32)
    template = np.asarray(template, f32)
    r1 = run_bass_kernel_spmd(_prog("p1"), host_prep_phase1(vertices),
                              list(range(8)), trace=trace)
    maps2, nbrs = host_prep_phase2(vertices, template, r1.results)
    r2 = run_bass_kernel_spmd(_prog("p2"), maps2, list(range(8)), trace=trace)
    if _timing is not None:
        _timing["phase1"] = r1
        _timing["phase2"] = r2
        _timing["maps2"] = maps2
        _timing["nbrs"] = nbrs
    return host_assemble(r2.results, nbrs)


if __name__ == "__main__":
    # Phase-1 standalone check against exact numpy KNN.
    cache = np.load("/root/problem/dev_cache/ref.npz")
    vertices = cache["vertices"]
    res = run_phase1(vertices)
    nbad = 0
    for core in range(8):
        b, h = core // 2, core % 2
        verts = vertices[b].astype(f32)
        Q = verts[h * HALF:(h + 1) * HALF]
        d2 = ((Q[:, None, :] - verts[None, :, :]) ** 2).sum(-1)
        ref_order = np.argsort(d2, axis=1, kind="stable")[:, :33]
        nbr, d, rad = host_merge(res.results[core]["candv"])
        rnbr = ref_order[:, :32]
        idx_match = (np.sort(nbr, 1) == np.sort(rnbr, 1)).mean()
        rrad = np.sqrt(np.take_along_axis(d2, ref_order[:, 32:33], axis=1)[:, 0])
        print(f"core {core}: top32 set match={idx_match:.6f} "
              f"rad maxdiff={np.abs(rad-rrad).max():.2e}")
        nbad += (np.sort(nbr, 1) != np.sort(rnbr, 1)).sum()
    print("total nbr mismatches vs exact:", nbad)

